# revision 1
# baseline (speedup 1.0000x reference)
"""Trainium2 Bass kernel for nn_Discriminator (fed-back LSTM cell, 64 steps).

Math (per batch row b):
    gh      = h0 @ W_hh.T + b_ih + b_hh          (constant across steps)
    x~_0    = start_emb - fc_b
    gh'     = gh + W_ih @ fc_b                    (bias folding so every step
    x~_{t+1} = h_t @ fc_W.T                        is bias-free)
    gates_t = W_ih @ x~_t + gh'   -> i,f,g,o
    c_t = sig(f)*c0 + sig(i)*tanh(g);  h_t = sig(o)*tanh(c_t)
    out = softmax(h_63 @ final_W.T + final_b) = [sig(d), sig(-d)],
          d = (final_W[0]-final_W[1]) @ h_63 + (final_b[0]-final_b[1])

Layout: everything transposed (feature dim on SBUF partitions, batch on the
free dim) so x~ and h flow between matmuls with zero on-device transposes.
All operands bf16 (fp32 PSUM accumulation); measured end-to-end max rel err
vs the fp32 reference ~2e-3.

Sharding: batch 16384 -> 2048 per core across 8 cores (data parallel, no
collectives). Each core runs 2 sequential half-batch passes of 1024 columns
so the gh' tensor (32x[128,1024] bf16 = 64KB/partition) stays SBUF-resident.
"""
import numpy as np
import ml_dtypes

import concourse.bass as bass
import concourse.tile as tile
from concourse import mybir
from concourse.bass_utils import run_bass_kernel_spmd

NPBF = ml_dtypes.bfloat16
NPF8 = ml_dtypes.float8_e4m3
BF16 = mybir.dt.bfloat16
F32 = mybir.dt.float32
FP8 = mybir.dt.float8e4
AF = mybir.ActivationFunctionType

B, E, H = 16384, 512, 1024
SEQ = 64
N_CORES = 8
BL = B // N_CORES          # 2048 batch per core
PASSES = 2
BP = BL // PASSES          # 1024 batch per pass
NT = 512                   # matmul moving-operand free dim
NB = BP // NT              # n-chunks per pass
KE = E // 128              # 4  k-chunks of E
KH = H // 128              # 8  k-chunks of H
MG = 4 * H // 128          # 32 m-chunks of 4H
PRELOAD_J = 0              # h-slices >= this use PE identity-preload for gh

TRACE = False              # set by test.py for profiling runs
TRACE_KWARGS = {}

# ---------------------------------------------------------------------------
# BIR post-pass: this container's walrus accepts at most ONE sync-wait command
# per instruction; Tile emits multi-sem waits. Split the excess onto NoOps.
# ---------------------------------------------------------------------------


def _split_sync_waits(bir: dict, limit: int = 1) -> int:
    n_nops = 0
    for fn in bir["functions"]:
        for bb in fn["blocks"]:
            insts = bb.get("instructions")
            if not insts:
                continue
            out = []
            for ins in insts:
                si = ins.get("sync_info")
                waits = (si or {}).get("on_wait") or []
                if len(waits) > limit:
                    imm = [w for w in waits if "imm" in str(w.get("wait_mode", ""))]
                    reg = [w for w in waits if "imm" not in str(w.get("wait_mode", ""))]
                    keep_n = max(0, limit - len(reg))
                    keep = reg + imm[:keep_n]
                    move = imm[keep_n:]
                    for i in range(0, len(move), limit):
                        out.append({
                            "debug": ins.get("debug", 0),
                            "engine": ins["engine"],
                            "ins": [],
                            "name": f"{ins['name']}-wsp{n_nops}",
                            "opcode": "NoOp",
                            "outs": [],
                            "sync_info": {"on_update": [],
                                          "on_wait": move[i:i + limit]},
                        })
                        n_nops += 1
                    si["on_wait"] = keep
                out.append(ins)
            bb["instructions"] = out
    return n_nops


def _install_wait_split_hook(limit: int = 1):
    import orjson

    if getattr(bass.Bass, "_wait_split_installed", False):
        return
    orig_str = bass.Bass.to_json_str
    orig_bytes = bass.Bass.to_json_bytes

    def _rewrite(raw):
        d = orjson.loads(raw)
        _split_sync_waits(d, limit=limit)
        return orjson.dumps(d)

    bass.Bass.to_json_str = lambda self, *a, **k: _rewrite(
        orig_str(self, *a, **k)).decode()
    bass.Bass.to_json_bytes = lambda self, *a, **k: _rewrite(
        orig_bytes(self, *a, **k))
    bass.Bass._wait_split_installed = True


# ---------------------------------------------------------------------------
# Device program
# ---------------------------------------------------------------------------


def _build_bass(seq: int = SEQ, unroll_loop: bool = False,
                passes: int = PASSES) -> bass.Bass:
    from contextlib import ExitStack

    nc = bass.Bass()
    x0T = nc.declare_dram_parameter("x0T", [128, KE, BL], FP8, isOutput=False)
    h0T = nc.declare_dram_parameter("h0T", [H, BL], BF16, isOutput=False)
    c0T = nc.declare_dram_parameter("c0T", [H, BL], BF16, isOutput=False)
    wih8 = nc.declare_dram_parameter("wih8", [128, KE, 4 * H], FP8, isOutput=False)
    whhT = nc.declare_dram_parameter("whhT", [H, 4 * H], BF16, isOutput=False)
    fcwT = nc.declare_dram_parameter("fcwT", [H, E], BF16, isOutput=False)
    biasv = nc.declare_dram_parameter("biasv", [4 * H], F32, isOutput=False)
    wdiff = nc.declare_dram_parameter("wdiff", [H], BF16, isOutput=False)
    biasd = nc.declare_dram_parameter("biasd", [1, 2], F32, isOutput=False)
    ident = nc.declare_dram_parameter("ident", [128, 128], BF16, isOutput=False)
    out = nc.declare_dram_parameter("out", [2, BL], F32, isOutput=True)

    gates = ("i", "f", "g", "o")
    gate_fn = {"i": AF.Sigmoid, "f": AF.Sigmoid, "g": AF.Tanh, "o": AF.Sigmoid}

    with tile.TileContext(nc) as tc, ExitStack() as gctx:
        const = gctx.enter_context(tc.tile_pool(name="const", bufs=1))
        bias_sb = const.tile([128, MG], F32, name="bias_sb", tag="bias_sb")
        nc.sync.dma_start(out=bias_sb, in_=biasv[:].rearrange("(m p) -> p m", p=128))
        wd_sb = const.tile([128, KH], BF16, name="wd_sb", tag="wd_sb")
        nc.sync.dma_start(out=wd_sb, in_=wdiff[:].rearrange("(k p) -> p k", p=128))
        bd_sb = const.tile([1, 2], F32, name="bd_sb", tag="bd_sb")
        nc.sync.dma_start(out=bd_sb, in_=biasd[:, :])
        id_sb = const.tile([128, 128], BF16, name="id_sb", tag="id_sb")
        nc.sync.dma_start(out=id_sb, in_=ident[:, :])

        for p in range(passes):
            bs = slice(p * BP, (p + 1) * BP)
            with ExitStack() as pctx:
                # --- pass-resident state ---
                ghp = pctx.enter_context(tc.tile_pool(name=f"gh{p}", bufs=1))
                c0p = pctx.enter_context(tc.tile_pool(name=f"c0{p}", bufs=1))
                xp = pctx.enter_context(tc.tile_pool(name=f"x{p}", bufs=1))
                gh = [ghp.tile([128, BP], BF16, name=f"gh{p}_{m}", tag=f"gh{m}")
                      for m in range(MG)]
                c0t = [c0p.tile([128, BP], BF16, name=f"c0{p}_{j}", tag=f"c0{j}")
                       for j in range(KH)]
                xt = xp.tile([128, KE, BP], FP8, name=f"x{p}", tag="x")
                # --- phase B: gh' = W_hh @ h0T + bias (scoped: frees W_hh) ---
                with ExitStack() as bctx:
                    whhp = bctx.enter_context(tc.tile_pool(name=f"whh{p}", bufs=1))
                    h0p = bctx.enter_context(tc.tile_pool(name=f"h0{p}", bufs=1))
                    pghp = bctx.enter_context(
                        tc.tile_pool(name=f"pgh{p}", bufs=1, space="PSUM"))
                    whh_sb = [whhp.tile([128, 4 * H], BF16, name=f"whh{p}_{k}",
                                        tag=f"whh{k}") for k in range(KH)]
                    h0_sb = [h0p.tile([128, BP], BF16, name=f"h0{p}_{k}",
                                      tag=f"h0{k}") for k in range(KH)]
                    HALF = 2 * H
                    for k in range(KH):
                        nc.sync.dma_start(out=whh_sb[k][:, :HALF],
                                          in_=whhT[k * 128:(k + 1) * 128, :HALF])
                        nc.gpsimd.dma_start(out=whh_sb[k][:, HALF:],
                                            in_=whhT[k * 128:(k + 1) * 128, HALF:])
                        (nc.sync if k % 2 else nc.gpsimd).dma_start(
                            out=h0_sb[k], in_=h0T[k * 128:(k + 1) * 128, bs])
                    for j in range(KH):
                        nc.sync.dma_start(out=c0t[j],
                                          in_=c0T[j * 128:(j + 1) * 128, bs])
                    nc.sync.dma_start(out=xt, in_=x0T[:, :, bs])
                    for m in range(MG):
                        ps = pghp.tile([128, BP], F32, name=f"pgh{p}_{m}",
                                       tag="pgh", bufs=4)
                        for k in range(KH):
                            for n in range(NB):
                                nc.tensor.matmul(
                                    ps[:, n * NT:(n + 1) * NT],
                                    lhsT=whh_sb[k][:, m * 128:(m + 1) * 128],
                                    rhs=h0_sb[k][:, n * NT:(n + 1) * NT],
                                    start=(k == 0), stop=(k == KH - 1))
                        if m % 2 == 0:
                            nc.vector.tensor_copy(gh[m], ps)
                        else:
                            nc.scalar.activation(gh[m], ps, AF.Copy)

                # --- main pools ---
                wp = pctx.enter_context(tc.tile_pool(name=f"wih{p}", bufs=1))
                fp_ = pctx.enter_context(tc.tile_pool(name=f"fcw{p}", bufs=1))
                hp = pctx.enter_context(tc.tile_pool(name=f"h{p}", bufs=1))
                work = pctx.enter_context(tc.tile_pool(name=f"work{p}", bufs=2))
                ps1p = pctx.enter_context(
                    tc.tile_pool(name=f"ps1{p}", bufs=2, space="PSUM"))
                ps2p = pctx.enter_context(
                    tc.tile_pool(name=f"ps2{p}", bufs=2, space="PSUM"))

                wih_sb = wp.tile([128, KE, 4 * H], FP8, name=f"wih{p}",
                                 tag="wih")
                fcw_sb = [fp_.tile([128, E], BF16, name=f"fcw{p}_{k}",
                                   tag=f"fcw{k}") for k in range(KH)]
                h_sb = [hp.tile([128, BP], BF16, name=f"h{p}_{j}", tag=f"h{j}")
                        for j in range(KH)]
                nc.sync.dma_start(out=wih_sb, in_=wih8[:, :, :])
                for k in range(KH):
                    nc.sync.dma_start(out=fcw_sb[k],
                                      in_=fcwT[k * 128:(k + 1) * 128, :])

                # --- 64-step recurrence ---
                # Emission is software-pipelined: the elementwise c/h chain
                # for slice j-1 is emitted between slice j's gate groups so
                # the static per-engine instruction order never stalls on a
                # cross-engine dependency that was issued immediately before.
                # t2/h-mul run on the otherwise-idle GPSIMD.
                def emit_gates(j, pend=()):
                    # j >= PRELOAD_J: gh is preloaded into PSUM by an
                    # identity matmul (PE) and sigma reads PSUM directly;
                    # else gh is added on DVE into a bf16 staging tile.
                    preload = j >= PRELOAD_J
                    pend = list(pend)
                    sig = {}
                    for g in gates:
                        if pend:
                            pend.pop(0)()
                        m = gates.index(g) * KH + j
                        ps = ps1p.tile([128, BP], F32, name=f"ps1_{j}{g}",
                                       tag="ps1", bufs=2)
                        if preload:
                            for n in range(NB):
                                nc.tensor.matmul(
                                    ps[:, n * NT:(n + 1) * NT],
                                    lhsT=id_sb,
                                    rhs=gh[m][:, n * NT:(n + 1) * NT],
                                    start=True, stop=False)
                        for s in range(0, KE, 2):
                            for n in range(NB):
                                nc.tensor.matmul(
                                    ps[:, n * NT:(n + 1) * NT],
                                    lhsT=wih_sb[:, s:s + 2,
                                                m * 128:(m + 1) * 128],
                                    rhs=xt[:, s:s + 2, n * NT:(n + 1) * NT],
                                    start=(s == 0 and not preload),
                                    stop=(s == KE - 2),
                                    perf_mode=mybir.MatmulPerfMode.DoubleRow)
                        s = work.tile([128, BP], BF16, name=f"sig_{j}{g}",
                                      tag=f"sig{g}", bufs=4)
                        if preload:
                            nc.scalar.activation(s, ps, gate_fn[g],
                                                 bias=bias_sb[:, m:m + 1])
                        else:
                            pre = work.tile([128, BP], BF16, name=f"pre_{j}{g}",
                                            tag="pre", bufs=4)
                            nc.vector.tensor_add(pre, ps, gh[m])
                            nc.scalar.activation(s, pre, gate_fn[g],
                                                 bias=bias_sb[:, m:m + 1])
                        sig[g] = s
                    return sig

                def cpath_pieces(j, sig):
                    """Yield the c/h chain for slice j as 4 pieces, to be
                    interleaved between the next slice's gate groups so no
                    engine's in-order stream stalls on a fresh dependency."""
                    t1 = work.tile([128, BP], BF16, name=f"t1_{j}",
                                   tag="t1", bufs=3)
                    t2 = work.tile([128, BP], BF16, name=f"t2_{j}",
                                   tag="t2", bufs=3)
                    cc = work.tile([128, BP], BF16, name=f"cc_{j}",
                                   tag="cc", bufs=3)
                    tch = work.tile([128, BP], BF16, name=f"tch_{j}",
                                    tag="tch", bufs=3)

                    def p0():
                        nc.vector.tensor_mul(t1, sig["f"], c0t[j])

                    def p1():
                        nc.vector.tensor_mul(t2, sig["i"], sig["g"])

                    def p2():
                        nc.vector.tensor_add(cc, t1, t2)
                        nc.scalar.activation(tch, cc, AF.Tanh)

                    def p3():
                        nc.vector.tensor_mul(h_sb[j], sig["o"], tch)

                    return [p0, p1, p2, p3]

                def mm2_partial(ms, klo, khi, pss):
                    for i, m in enumerate(ms):
                        for k in range(klo, khi):
                            for n in range(NB):
                                nc.tensor.matmul(
                                    pss[i][:, n * NT:(n + 1) * NT],
                                    lhsT=fcw_sb[k][:, m * 128:(m + 1) * 128],
                                    rhs=h_sb[k][:, n * NT:(n + 1) * NT],
                                    start=(k == klo), stop=(k == khi - 1))

                def mm2_finish(ms, pss):
                    for i, m in enumerate(ms):
                        k = KH - 1
                        for n in range(NB):
                            nc.tensor.matmul(
                                pss[i][:, n * NT:(n + 1) * NT],
                                lhsT=fcw_sb[k][:, m * 128:(m + 1) * 128],
                                rhs=h_sb[k][:, n * NT:(n + 1) * NT],
                                start=False, stop=True)
                    for i, m in enumerate(ms):
                        if m % 2 == 0:
                            nc.vector.tensor_copy(xt[:, m, :], pss[i])
                        else:
                            nc.scalar.activation(xt[:, m, :], pss[i], AF.Copy)

                def step_body():
                    pend = []
                    for j in range(KH):
                        sig = emit_gates(j, pend)
                        pend = cpath_pieces(j, sig)
                        if j == KH - 2:
                            # m2/m3: accumulate k=0..KH-3 now (h_0..h_5
                            # ready), park the partial in SBUF so the PSUM
                            # slots free up; the tail only needs k>=KH-2.
                            pss1 = [ps2p.tile([128, BP], F32, name=f"ps2_{m}",
                                              tag="ps2", bufs=2)
                                    for m in (2, 3)]
                            mm2_partial((2, 3), 0, KH - 2, pss1)
                            xparts = []
                            for i, m in enumerate((2, 3)):
                                xp_ = work.tile([128, BP], BF16,
                                                name=f"xpart_{m}",
                                                tag=f"xpart{i}", bufs=1)
                                nc.vector.tensor_copy(xp_, pss1[i])
                                xparts.append(xp_)
                    # pair 0: k=0..KH-2 accumulates while the last slice's
                    # c/h chain is in flight; h_{KH-1} finishers after.
                    pss0 = [ps2p.tile([128, BP], F32, name=f"ps2_{m}",
                                      tag="ps2", bufs=2) for m in (0, 1)]
                    mm2_partial((0, 1), 0, KH - 1, pss0)
                    for piece in pend:
                        piece()
                    mm2_finish((0, 1), pss0)
                    # pair 1 tail: k=KH-2..KH-1 into fresh psum + SBUF partial
                    pss1b = [ps2p.tile([128, BP], F32, name=f"ps2b_{m}",
                                       tag="ps2", bufs=2) for m in (2, 3)]
                    for i, m in enumerate((2, 3)):
                        for k in (KH - 2, KH - 1):
                            for n in range(NB):
                                nc.tensor.matmul(
                                    pss1b[i][:, n * NT:(n + 1) * NT],
                                    lhsT=fcw_sb[k][:, m * 128:(m + 1) * 128],
                                    rhs=h_sb[k][:, n * NT:(n + 1) * NT],
                                    start=(k == KH - 2), stop=(k == KH - 1))
                        nc.vector.tensor_add(xt[:, m, :], pss1b[i], xparts[i])

                if unroll_loop:
                    for _ in range(seq):
                        step_body()
                else:
                    assert seq % 8 == 0
                    with tc.For_i(0, seq, 8,
                                  hint_engines=(mybir.EngineType.PE,
                                                mybir.EngineType.DVE,
                                                mybir.EngineType.Activation)):
                        for _ in range(8):
                            step_body()

                # --- head: d = wdiff @ h_63; p0 = sig(d+bd), p1 = sig(-d-bd) ---
                psd = ps2p.tile([1, BP], F32, name=f"psd{p}", tag="ps2", bufs=2)
                for n in range(NB):
                    for k in range(KH):
                        nc.tensor.matmul(
                            psd[0:1, n * NT:(n + 1) * NT],
                            lhsT=wd_sb[:, k:k + 1],
                            rhs=h_sb[k][:, n * NT:(n + 1) * NT],
                            start=(k == 0), stop=(k == KH - 1))
                p0 = work.tile([1, BP], F32, name=f"p0_{p}", tag="p0", bufs=1)
                p1 = work.tile([1, BP], F32, name=f"p1_{p}", tag="p1", bufs=1)
                nc.scalar.activation(p0, psd, AF.Sigmoid,
                                     bias=bd_sb[0:1, 0:1], scale=1.0)
                nc.scalar.activation(p1, psd, AF.Sigmoid,
                                     bias=bd_sb[0:1, 1:2], scale=-1.0)
                nc.sync.dma_start(out=out[0:1, bs], in_=p0)
                nc.sync.dma_start(out=out[1:2, bs], in_=p1)
    return nc


# ---------------------------------------------------------------------------
# Host wrapper
# ---------------------------------------------------------------------------


def kernel(start_emb, h0, c0, W_ih, W_hh, b_ih, b_hh, fc_W, fc_b,
           final_W, final_b):
    _install_wait_split_hook()

    start_emb = np.asarray(start_emb, np.float32)
    h0 = np.asarray(h0, np.float32)
    c0 = np.asarray(c0, np.float32)
    W_ih = np.asarray(W_ih, np.float32)
    W_hh = np.asarray(W_hh, np.float32)
    b_ih = np.asarray(b_ih, np.float32)
    b_hh = np.asarray(b_hh, np.float32)
    fc_W = np.asarray(fc_W, np.float32)
    fc_b = np.asarray(fc_b, np.float32)
    final_W = np.asarray(final_W, np.float32)
    final_b = np.asarray(final_b, np.float32)

    # shared (replicated) weight prep, all layout work on host
    wih8 = np.ascontiguousarray(
        W_ih.T.reshape(KE, 128, 4 * H).transpose(1, 0, 2)).astype(NPF8)
    whhT = np.ascontiguousarray(W_hh.T).astype(NPBF)              # [H, 4H]
    fcwT = np.ascontiguousarray(fc_W.T).astype(NPBF)              # [H, E]
    biasv = (b_ih + b_hh + W_ih @ fc_b).astype(np.float32)        # [4H]
    wdiff = (final_W[0] - final_W[1]).astype(NPBF)                # [H]
    bd = float(final_b[0]) - float(final_b[1])
    biasd = np.array([[bd, -bd]], np.float32)
    identity = np.eye(128, dtype=NPBF)

    x0 = start_emb[:, 0, :] - fc_b                                # [B, E]
    x0T8 = np.ascontiguousarray(
        x0.T.reshape(KE, 128, B).transpose(1, 0, 2)).astype(NPF8)
    h0s = h0[0]                                                   # [B, H]
    c0s = c0[0]                                                   # [B, H]

    in_maps = []
    for ci in range(N_CORES):
        sl = slice(ci * BL, (ci + 1) * BL)
        in_maps.append({
            "x0T": np.ascontiguousarray(x0T8[:, :, sl]),
            "h0T": np.ascontiguousarray(h0s[sl].T).astype(NPBF),
            "c0T": np.ascontiguousarray(c0s[sl].T).astype(NPBF),
            "wih8": wih8,
            "whhT": whhT,
            "fcwT": fcwT,
            "biasv": biasv,
            "wdiff": wdiff,
            "biasd": biasd,
            "ident": identity,
        })

    nc = _build_bass()
    kernel.last_nc = nc
    import time as _time
    t0 = _time.monotonic()
    res = run_bass_kernel_spmd(nc, in_maps, list(range(N_CORES)),
                               trace=TRACE, **TRACE_KWARGS)
    kernel.last_wall_s = _time.monotonic() - t0
    kernel.last_results = res

    full = np.empty((B, 1, 2), np.float32)
    for ci in range(N_CORES):
        o = res.results[ci]["out"]                                # [2, BL]
        full[ci * BL:(ci + 1) * BL, 0, 0] = o[0]
        full[ci * BL:(ci + 1) * BL, 0, 1] = o[1]
    return full



# revision 2
# speedup vs baseline: 18.5094x; 18.5094x over previous
"""Trainium2 Bass kernel for nn_Discriminator (fed-back LSTM cell, 64 steps).

Math (per batch row b):
    gh      = h0 @ W_hh.T + b_ih + b_hh          (constant across steps)
    x~_0    = start_emb - fc_b
    gh'     = gh + W_ih @ fc_b                    (bias folding so every step
    x~_{t+1} = h_t @ fc_W.T                        is bias-free)
    gates_t = W_ih @ x~_t + gh'   -> i,f,g,o
    c_t = sig(f)*c0 + sig(i)*tanh(g);  h_t = sig(o)*tanh(c_t)
    out = softmax(h_63 @ final_W.T + final_b) = [sig(d), sig(-d)],
          d = (final_W[0]-final_W[1]) @ h_63 + (final_b[0]-final_b[1])

Layout: everything transposed (feature dim on SBUF partitions, batch on the
free dim) so x~ and h flow between matmuls with zero on-device transposes.

Engine budget per step (BP=1024 batch columns), from the TRN2 cost model:
the Activation engine owns all 40 transcendental slice-ops (32 gate
sigmoid/tanh + 8 tanh(c)) at ~1.04us each and is the binding engine
(~41.5us busy). Everything else is kept strictly below that: PE ~30.7us
(gates fp8 DoubleRow + bf16 identity-preload of gh + fc in fp8 DoubleRow),
DVE ~23us (c-chain muls/adds at the 2x bf16 rate + fp8 converts), Pool
(GPSIMD) absorbs ~16us of muls/converts. The fc output staging is
software-pipelined in two waves so only ~0.4us of PE work and the xt
converts remain after the last h-slice of a step.

Sharding: batch 16384 -> 2048 per core across 8 cores (data parallel, no
collectives). Each core runs 2 sequential half-batch passes of 1024 columns
so the gh' tensor (32x[128,1024] bf16 = 64KB/partition) stays SBUF-resident.
"""
import numpy as np
import ml_dtypes

import concourse.bass as bass
import concourse.tile as tile
from concourse import mybir
from concourse.bass_utils import run_bass_kernel_spmd

NPBF = ml_dtypes.bfloat16
NPF8 = ml_dtypes.float8_e4m3
BF16 = mybir.dt.bfloat16
F32 = mybir.dt.float32
FP8 = mybir.dt.float8e4
AF = mybir.ActivationFunctionType
DR = mybir.MatmulPerfMode.DoubleRow

B, E, H = 16384, 512, 1024
SEQ = 64
N_CORES = 8
BL = B // N_CORES          # 2048 batch per core
PASSES = 2
BP = BL // PASSES          # 1024 batch per pass
NT = 512                   # matmul moving-operand free dim
NB = BP // NT              # n-chunks per pass
KE = E // 128              # 4  k-chunks of E
KH = H // 128              # 8  k-chunks of H
MG = 4 * H // 128          # 32 m-chunks of 4H

TRACE = False              # set by test.py for profiling runs
TRACE_KWARGS = {}

# ---------------------------------------------------------------------------
# BIR post-pass: this container's walrus accepts at most ONE sync-wait command
# per instruction; Tile emits multi-sem waits. Split the excess onto NoOps.
# ---------------------------------------------------------------------------


def _split_sync_waits(bir: dict, limit: int = 1) -> int:
    n_nops = 0
    for fn in bir["functions"]:
        for bb in fn["blocks"]:
            insts = bb.get("instructions")
            if not insts:
                continue
            out = []
            for ins in insts:
                si = ins.get("sync_info")
                waits = (si or {}).get("on_wait") or []
                if len(waits) > limit:
                    imm = [w for w in waits if "imm" in str(w.get("wait_mode", ""))]
                    reg = [w for w in waits if "imm" not in str(w.get("wait_mode", ""))]
                    keep_n = max(0, limit - len(reg))
                    keep = reg + imm[:keep_n]
                    move = imm[keep_n:]
                    for i in range(0, len(move), limit):
                        out.append({
                            "debug": ins.get("debug", 0),
                            "engine": ins["engine"],
                            "ins": [],
                            "name": f"{ins['name']}-wsp{n_nops}",
                            "opcode": "NoOp",
                            "outs": [],
                            "sync_info": {"on_update": [],
                                          "on_wait": move[i:i + limit]},
                        })
                        n_nops += 1
                    si["on_wait"] = keep
                out.append(ins)
            bb["instructions"] = out
    return n_nops


def _install_wait_split_hook(limit: int = 1):
    import orjson

    if getattr(bass.Bass, "_wait_split_installed", False):
        return
    orig_str = bass.Bass.to_json_str
    orig_bytes = bass.Bass.to_json_bytes

    def _rewrite(raw):
        d = orjson.loads(raw)
        _split_sync_waits(d, limit=limit)
        return orjson.dumps(d)

    bass.Bass.to_json_str = lambda self, *a, **k: _rewrite(
        orig_str(self, *a, **k)).decode()
    bass.Bass.to_json_bytes = lambda self, *a, **k: _rewrite(
        orig_bytes(self, *a, **k))
    bass.Bass._wait_split_installed = True


# ---------------------------------------------------------------------------
# Device program
# ---------------------------------------------------------------------------


def _build_bass(seq: int = SEQ, unroll_loop: bool = False,
                passes: int = PASSES) -> bass.Bass:
    from contextlib import ExitStack

    nc = bass.Bass()
    x0T = nc.declare_dram_parameter("x0T", [128, KE, BL], FP8, isOutput=False)
    h0T = nc.declare_dram_parameter("h0T", [H, BL], BF16, isOutput=False)
    c0T = nc.declare_dram_parameter("c0T", [H, BL], BF16, isOutput=False)
    wih8 = nc.declare_dram_parameter("wih8", [128, KE, 4 * H], FP8, isOutput=False)
    whhT = nc.declare_dram_parameter("whhT", [H, 4 * H], BF16, isOutput=False)
    fcw8d = nc.declare_dram_parameter("fcw8", [128, KH, E], FP8, isOutput=False)
    biasv = nc.declare_dram_parameter("biasv", [4 * H], F32, isOutput=False)
    wdiff = nc.declare_dram_parameter("wdiff", [H], BF16, isOutput=False)
    biasd = nc.declare_dram_parameter("biasd", [1, 2], F32, isOutput=False)
    ident = nc.declare_dram_parameter("ident", [128, 128], BF16, isOutput=False)
    out = nc.declare_dram_parameter("out", [2, BL], F32, isOutput=True)

    gates = ("i", "f", "g", "o")
    gate_fn = {"i": AF.Sigmoid, "f": AF.Sigmoid, "g": AF.Tanh, "o": AF.Sigmoid}

    with tile.TileContext(nc) as tc, ExitStack() as gctx:
        const = gctx.enter_context(tc.tile_pool(name="const", bufs=1))
        bias_sb = const.tile([128, MG], F32, name="bias_sb", tag="bias_sb")
        nc.sync.dma_start(out=bias_sb, in_=biasv[:].rearrange("(m p) -> p m", p=128))
        wd_sb = const.tile([128, KH], BF16, name="wd_sb", tag="wd_sb")
        nc.sync.dma_start(out=wd_sb, in_=wdiff[:].rearrange("(k p) -> p k", p=128))
        bd_sb = const.tile([1, 2], F32, name="bd_sb", tag="bd_sb")
        nc.sync.dma_start(out=bd_sb, in_=biasd[:, :])
        id_sb = const.tile([128, 128], BF16, name="id_sb", tag="id_sb")
        nc.sync.dma_start(out=id_sb, in_=ident[:, :])

        for p in range(passes):
            bs = slice(p * BP, (p + 1) * BP)
            with ExitStack() as pctx:
                # --- pass-resident state ---
                ghp = pctx.enter_context(tc.tile_pool(name=f"gh{p}", bufs=1))
                c0p = pctx.enter_context(tc.tile_pool(name=f"c0{p}", bufs=1))
                xp = pctx.enter_context(tc.tile_pool(name=f"x{p}", bufs=1))
                gh = [ghp.tile([128, BP], BF16, name=f"gh{p}_{m}", tag=f"gh{m}")
                      for m in range(MG)]
                c0t = [c0p.tile([128, BP], BF16, name=f"c0{p}_{j}", tag=f"c0{j}")
                       for j in range(KH)]
                xt = xp.tile([128, KE, BP], FP8, name=f"x{p}", tag="x")
                # --- phase B: gh' = W_hh @ h0T + bias (scoped: frees W_hh) ---
                with ExitStack() as bctx:
                    whhp = bctx.enter_context(tc.tile_pool(name=f"whh{p}", bufs=1))
                    h0p = bctx.enter_context(tc.tile_pool(name=f"h0{p}", bufs=1))
                    pghp = bctx.enter_context(
                        tc.tile_pool(name=f"pgh{p}", bufs=1, space="PSUM"))
                    whh_sb = [whhp.tile([128, 4 * H], BF16, name=f"whh{p}_{k}",
                                        tag=f"whh{k}") for k in range(KH)]
                    h0_sb = [h0p.tile([128, BP], BF16, name=f"h0{p}_{k}",
                                      tag=f"h0{k}") for k in range(KH)]
                    HALF = 2 * H
                    for k in range(KH):
                        nc.sync.dma_start(out=whh_sb[k][:, :HALF],
                                          in_=whhT[k * 128:(k + 1) * 128, :HALF])
                        nc.gpsimd.dma_start(out=whh_sb[k][:, HALF:],
                                            in_=whhT[k * 128:(k + 1) * 128, HALF:])
                        (nc.sync if k % 2 else nc.gpsimd).dma_start(
                            out=h0_sb[k], in_=h0T[k * 128:(k + 1) * 128, bs])
                    for j in range(KH):
                        nc.sync.dma_start(out=c0t[j],
                                          in_=c0T[j * 128:(j + 1) * 128, bs])
                    nc.sync.dma_start(out=xt, in_=x0T[:, :, bs])
                    for m in range(MG):
                        ps = pghp.tile([128, BP], F32, name=f"pgh{p}_{m}",
                                       tag="pgh", bufs=4)
                        for k in range(KH):
                            for n in range(NB):
                                nc.tensor.matmul(
                                    ps[:, n * NT:(n + 1) * NT],
                                    lhsT=whh_sb[k][:, m * 128:(m + 1) * 128],
                                    rhs=h0_sb[k][:, n * NT:(n + 1) * NT],
                                    start=(k == 0), stop=(k == KH - 1))
                        if m % 2 == 0:
                            nc.vector.tensor_copy(gh[m], ps)
                        else:
                            nc.scalar.activation(gh[m], ps, AF.Copy)

                # --- main pools ---
                wp = pctx.enter_context(tc.tile_pool(name=f"wih{p}", bufs=1))
                fp_ = pctx.enter_context(tc.tile_pool(name=f"fcw{p}", bufs=1))
                hp = pctx.enter_context(tc.tile_pool(name=f"h{p}", bufs=1))
                work = pctx.enter_context(tc.tile_pool(name=f"work{p}", bufs=2))
                ps1p = pctx.enter_context(
                    tc.tile_pool(name=f"ps1{p}", bufs=2, space="PSUM"))
                ps2p = pctx.enter_context(
                    tc.tile_pool(name=f"ps2{p}", bufs=2, space="PSUM"))

                wih_sb = wp.tile([128, KE, 4 * H], FP8, name=f"wih{p}",
                                 tag="wih")
                fcw8 = fp_.tile([128, KH, E], FP8, name=f"fcw{p}", tag="fcw")
                h_sb = [hp.tile([128, BP], BF16, name=f"h{p}_{j}", tag=f"h{j}")
                        for j in range(KH)]
                h8 = hp.tile([128, KH, BP], FP8, name=f"h8{p}", tag="h8")
                nc.sync.dma_start(out=wih_sb, in_=wih8[:, :, :])
                nc.sync.dma_start(out=fcw8, in_=fcw8d[:, :, :])

                # --- 64-step recurrence ---
                # Emission is software-pipelined: the elementwise c/h chain
                # for slice j-1 is emitted between slice j's gate groups so
                # the static per-engine instruction order never stalls on a
                # cross-engine dependency that was issued immediately before.
                def emit_gates(j, pend=()):
                    # gh is preloaded into PSUM by an identity matmul (PE)
                    # and the sigma/tanh reads PSUM directly with the bias
                    # applied on the Activation engine.
                    pend = list(pend)
                    sig = {}
                    for g in gates:
                        if pend:
                            pend.pop(0)()
                        m = gates.index(g) * KH + j
                        ps = ps1p.tile([128, BP], F32, name=f"ps1_{j}{g}",
                                       tag="ps1", bufs=2)
                        for n in range(NB):
                            nc.tensor.matmul(
                                ps[:, n * NT:(n + 1) * NT],
                                lhsT=id_sb,
                                rhs=gh[m][:, n * NT:(n + 1) * NT],
                                start=True, stop=False)
                        for s in range(0, KE, 2):
                            for n in range(NB):
                                nc.tensor.matmul(
                                    ps[:, n * NT:(n + 1) * NT],
                                    lhsT=wih_sb[:, s:s + 2,
                                                m * 128:(m + 1) * 128],
                                    rhs=xt[:, s:s + 2, n * NT:(n + 1) * NT],
                                    start=False,
                                    stop=(s == KE - 2),
                                    perf_mode=DR)
                        s = work.tile([128, BP], BF16, name=f"sig_{j}{g}",
                                      tag=f"sig{g}", bufs=4)
                        nc.scalar.activation(s, ps, gate_fn[g],
                                             bias=bias_sb[:, m:m + 1])
                        sig[g] = s
                    for piece in pend:
                        piece()
                    return sig

                def cpath_pieces(j, sig):
                    """Yield the c/h chain for slice j as 4 pieces, to be
                    interleaved between the next slice's gate groups so no
                    engine's in-order stream stalls on a fresh dependency."""
                    t1 = work.tile([128, BP], BF16, name=f"t1_{j}",
                                   tag="t1", bufs=3)
                    t2 = work.tile([128, BP], BF16, name=f"t2_{j}",
                                   tag="t2", bufs=3)
                    cc = work.tile([128, BP], BF16, name=f"cc_{j}",
                                   tag="cc", bufs=3)
                    tch = work.tile([128, BP], BF16, name=f"tch_{j}",
                                    tag="tch", bufs=3)

                    def p0():
                        nc.vector.tensor_mul(t1, sig["f"], c0t[j])

                    def p1():
                        nc.vector.tensor_mul(t2, sig["i"], sig["g"])

                    def p2():
                        nc.vector.tensor_add(cc, t1, t2)
                        nc.scalar.activation(tch, cc, AF.Tanh)

                    def p3():
                        # h in bf16 (head + cheap 2x DVE), fp8 copy for the
                        # DoubleRow fc matmul; engines alternate by slice.
                        if j % 2 == 0:
                            nc.vector.tensor_mul(h_sb[j], sig["o"], tch)
                            nc.gpsimd.tensor_copy(h8[:, j, :], h_sb[j])
                        else:
                            nc.gpsimd.tensor_mul(h_sb[j], sig["o"], tch)
                            nc.vector.tensor_copy(h8[:, j, :], h_sb[j])

                    return [p0, p1, p2, p3]

                def fc_pairs(ms, pairs, pss, first):
                    for i, m in enumerate(ms):
                        for k in pairs:
                            for n in range(NB):
                                nc.tensor.matmul(
                                    pss[i][:, n * NT:(n + 1) * NT],
                                    lhsT=fcw8[:, k:k + 2,
                                              m * 128:(m + 1) * 128],
                                    rhs=h8[:, k:k + 2, n * NT:(n + 1) * NT],
                                    start=(k == first),
                                    stop=(k == KH - 2),
                                    perf_mode=DR)

                def step_body():
                    pend = []
                    xparts = []
                    pss01 = None
                    for j in range(KH):
                        sig = emit_gates(j, pend)
                        pend = cpath_pieces(j, sig)
                        if j == 6:
                            # wave A: m2/m3 over h-pairs (0..5), parked to
                            # SBUF so the PSUM slots free up for wave B.
                            pssA = [ps2p.tile([128, BP], F32, name=f"ps2_{m}",
                                              tag="ps2", bufs=2)
                                    for m in (2, 3)]
                            fc_pairs((2, 3), (0, 2, 4), pssA, 0)
                            for i, m in enumerate((2, 3)):
                                xp_ = work.tile([128, BP], BF16,
                                                name=f"xpart_{m}",
                                                tag=f"xpart{i}", bufs=1)
                                (nc.vector if m == 2 else
                                 nc.gpsimd).tensor_copy(xp_, pssA[i])
                                xparts.append(xp_)
                        if j == 7:
                            # wave B: m0/m1 over h-pairs (0..5); the tiles
                            # stay live to take the (6,7) tail right after
                            # h_7 lands.
                            pss01 = [ps2p.tile([128, BP], F32, name=f"ps2_{m}",
                                               tag="ps2", bufs=2)
                                     for m in (0, 1)]
                            fc_pairs((0, 1), (0, 2, 4), pss01, 0)
                    for piece in pend:
                        piece()
                    # tails: only the (h6,h7) pair + converts remain after
                    # the last h slice. xt0 lands on ACT (idle here), xt1 on
                    # DVE; the parked m2/m3 rejoin via adds on DVE/Pool.
                    fc_pairs((0, 1), (6,), pss01, None)
                    nc.scalar.activation(xt[:, 0, :], pss01[0], AF.Copy)
                    nc.vector.tensor_copy(xt[:, 1, :], pss01[1])
                    pssB = [ps2p.tile([128, BP], F32, name=f"ps2b_{m}",
                                      tag="ps2", bufs=2) for m in (2, 3)]
                    for i, m in enumerate((2, 3)):
                        for n in range(NB):
                            nc.tensor.matmul(
                                pssB[i][:, n * NT:(n + 1) * NT],
                                lhsT=fcw8[:, 6:8, m * 128:(m + 1) * 128],
                                rhs=h8[:, 6:8, n * NT:(n + 1) * NT],
                                start=True, stop=True,
                                perf_mode=DR)
                    nc.vector.tensor_add(xt[:, 2, :], pssB[0], xparts[0])
                    nc.gpsimd.tensor_add(xt[:, 3, :], pssB[1], xparts[1])

                if unroll_loop:
                    for _ in range(seq):
                        step_body()
                else:
                    assert seq % 8 == 0
                    with tc.For_i(0, seq, 8,
                                  hint_engines=(mybir.EngineType.PE,
                                                mybir.EngineType.DVE,
                                                mybir.EngineType.Activation,
                                                mybir.EngineType.Pool)):
                        for _ in range(8):
                            step_body()

                # --- head: d = wdiff @ h_63; p0 = sig(d+bd), p1 = sig(-d-bd) ---
                psd = ps2p.tile([1, BP], F32, name=f"psd{p}", tag="ps2", bufs=2)
                for n in range(NB):
                    for k in range(KH):
                        nc.tensor.matmul(
                            psd[0:1, n * NT:(n + 1) * NT],
                            lhsT=wd_sb[:, k:k + 1],
                            rhs=h_sb[k][:, n * NT:(n + 1) * NT],
                            start=(k == 0), stop=(k == KH - 1))
                p0 = work.tile([1, BP], F32, name=f"p0_{p}", tag="p0", bufs=1)
                p1 = work.tile([1, BP], F32, name=f"p1_{p}", tag="p1", bufs=1)
                nc.scalar.activation(p0, psd, AF.Sigmoid,
                                     bias=bd_sb[0:1, 0:1], scale=1.0)
                nc.scalar.activation(p1, psd, AF.Sigmoid,
                                     bias=bd_sb[0:1, 1:2], scale=-1.0)
                nc.sync.dma_start(out=out[0:1, bs], in_=p0)
                nc.sync.dma_start(out=out[1:2, bs], in_=p1)
    return nc


# ---------------------------------------------------------------------------
# Host wrapper
# ---------------------------------------------------------------------------


def kernel(start_emb, h0, c0, W_ih, W_hh, b_ih, b_hh, fc_W, fc_b,
           final_W, final_b):
    _install_wait_split_hook()

    start_emb = np.asarray(start_emb, np.float32)
    h0 = np.asarray(h0, np.float32)
    c0 = np.asarray(c0, np.float32)
    W_ih = np.asarray(W_ih, np.float32)
    W_hh = np.asarray(W_hh, np.float32)
    b_ih = np.asarray(b_ih, np.float32)
    b_hh = np.asarray(b_hh, np.float32)
    fc_W = np.asarray(fc_W, np.float32)
    fc_b = np.asarray(fc_b, np.float32)
    final_W = np.asarray(final_W, np.float32)
    final_b = np.asarray(final_b, np.float32)

    # shared (replicated) weight prep, all layout work on host
    wih8 = np.ascontiguousarray(
        W_ih.T.reshape(KE, 128, 4 * H).transpose(1, 0, 2)).astype(NPF8)
    whhT = np.ascontiguousarray(W_hh.T).astype(NPBF)              # [H, 4H]
    fcw8 = np.ascontiguousarray(
        fc_W.T.reshape(KH, 128, E).transpose(1, 0, 2)).astype(NPF8)
    biasv = (b_ih + b_hh + W_ih @ fc_b).astype(np.float32)        # [4H]
    wdiff = (final_W[0] - final_W[1]).astype(NPBF)                # [H]
    bd = float(final_b[0]) - float(final_b[1])
    biasd = np.array([[bd, -bd]], np.float32)
    identity = np.eye(128, dtype=NPBF)

    x0 = start_emb[:, 0, :] - fc_b                                # [B, E]
    x0T8 = np.ascontiguousarray(
        x0.T.reshape(KE, 128, B).transpose(1, 0, 2)).astype(NPF8)
    h0s = h0[0]                                                   # [B, H]
    c0s = c0[0]                                                   # [B, H]

    in_maps = []
    for ci in range(N_CORES):
        sl = slice(ci * BL, (ci + 1) * BL)
        in_maps.append({
            "x0T": np.ascontiguousarray(x0T8[:, :, sl]),
            "h0T": np.ascontiguousarray(h0s[sl].T).astype(NPBF),
            "c0T": np.ascontiguousarray(c0s[sl].T).astype(NPBF),
            "wih8": wih8,
            "whhT": whhT,
            "fcw8": fcw8,
            "biasv": biasv,
            "wdiff": wdiff,
            "biasd": biasd,
            "ident": identity,
        })

    nc = _build_bass()
    kernel.last_nc = nc
    import time as _time
    t0 = _time.monotonic()
    res = run_bass_kernel_spmd(nc, in_maps, list(range(N_CORES)),
                               trace=TRACE, **TRACE_KWARGS)
    kernel.last_wall_s = _time.monotonic() - t0
    kernel.last_results = res

    full = np.empty((B, 1, 2), np.float32)
    for ci in range(N_CORES):
        o = res.results[ci]["out"]                                # [2, BL]
        full[ci * BL:(ci + 1) * BL, 0, 0] = o[0]
        full[ci * BL:(ci + 1) * BL, 0, 1] = o[1]
    return full


# revision 7
# speedup vs baseline: 19.8360x; 1.0717x over previous
"""Trainium2 Bass kernel for nn_Discriminator (fed-back LSTM cell, 64 steps).

Math (per batch row b):
    gh      = h0 @ W_hh.T + b_ih + b_hh          (constant across steps)
    x~_0    = start_emb - fc_b
    gh'     = gh + W_ih @ fc_b                    (bias folding so every step
    x~_{t+1} = h_t @ fc_W.T                        is bias-free)
    gates_t = W_ih @ x~_t + gh'   -> i,f,g,o
    c_t = sig(f)*c0 + sig(i)*tanh(g);  h_t = sig(o)*tanh(c_t)
    out = softmax(h_63 @ final_W.T + final_b) = [sig(d), sig(-d)],
          d = (final_W[0]-final_W[1]) @ h_63 + (final_b[0]-final_b[1])

Layout: everything transposed (feature dim on SBUF partitions, batch on the
free dim) so x~ and h flow between matmuls with zero on-device transposes.

Engine budget per step (BP=1024 batch columns), from the TRN2 cost model:
the Activation engine owns all 40 transcendental slice-ops (32 gate
sigmoid/tanh + 8 tanh(c)) at ~1.04us each and is the binding engine
(~41.5us busy). Everything else is kept strictly below that: PE ~30.7us
(gates fp8 DoubleRow + bf16 identity-preload of gh + fc in fp8 DoubleRow),
DVE ~23us (c-chain muls/adds at the 2x bf16 rate + fp8 converts), Pool
(GPSIMD) absorbs ~16us of muls/converts. The fc output staging is
software-pipelined in two waves so only ~0.4us of PE work and the xt
converts remain after the last h-slice of a step.

Sharding: batch 16384 -> 2048 per core across 8 cores (data parallel, no
collectives). Each core runs 2 sequential half-batch passes of 1024 columns
so the gh' tensor (32x[128,1024] bf16 = 64KB/partition) stays SBUF-resident.
"""
import numpy as np
import ml_dtypes

import concourse.bass as bass
import concourse.tile as tile
from concourse import mybir
from concourse.bass_utils import run_bass_kernel_spmd

NPBF = ml_dtypes.bfloat16
NPF8 = ml_dtypes.float8_e4m3
BF16 = mybir.dt.bfloat16
F32 = mybir.dt.float32
FP8 = mybir.dt.float8e4
AF = mybir.ActivationFunctionType
DR = mybir.MatmulPerfMode.DoubleRow

B, E, H = 16384, 512, 1024
SEQ = 64
N_CORES = 8
BL = B // N_CORES          # 2048 batch per core
PASSES = 2
BP = BL // PASSES          # 1024 batch per pass
NT = 512                   # matmul moving-operand free dim
NB = BP // NT              # n-chunks per pass
KE = E // 128              # 4  k-chunks of E
KH = H // 128              # 8  k-chunks of H
MG = 4 * H // 128          # 32 m-chunks of 4H

TRACE = False              # set by test.py for profiling runs
TRACE_KWARGS = {}

# ---------------------------------------------------------------------------
# BIR post-pass: this container's walrus accepts at most ONE sync-wait command
# per instruction; Tile emits multi-sem waits. Split the excess onto NoOps.
# ---------------------------------------------------------------------------


def _split_sync_waits(bir: dict, limit: int = 1) -> int:
    n_nops = 0
    for fn in bir["functions"]:
        for bb in fn["blocks"]:
            insts = bb.get("instructions")
            if not insts:
                continue
            out = []
            for ins in insts:
                si = ins.get("sync_info")
                waits = (si or {}).get("on_wait") or []
                if len(waits) > limit:
                    imm = [w for w in waits if "imm" in str(w.get("wait_mode", ""))]
                    reg = [w for w in waits if "imm" not in str(w.get("wait_mode", ""))]
                    keep_n = max(0, limit - len(reg))
                    keep = reg + imm[:keep_n]
                    move = imm[keep_n:]
                    for i in range(0, len(move), limit):
                        out.append({
                            "debug": ins.get("debug", 0),
                            "engine": ins["engine"],
                            "ins": [],
                            "name": f"{ins['name']}-wsp{n_nops}",
                            "opcode": "NoOp",
                            "outs": [],
                            "sync_info": {"on_update": [],
                                          "on_wait": move[i:i + limit]},
                        })
                        n_nops += 1
                    si["on_wait"] = keep
                out.append(ins)
            bb["instructions"] = out
    return n_nops


def _install_wait_split_hook(limit: int = 1):
    import orjson

    if getattr(bass.Bass, "_wait_split_installed", False):
        return
    orig_str = bass.Bass.to_json_str
    orig_bytes = bass.Bass.to_json_bytes

    def _rewrite(raw):
        d = orjson.loads(raw)
        _split_sync_waits(d, limit=limit)
        return orjson.dumps(d)

    bass.Bass.to_json_str = lambda self, *a, **k: _rewrite(
        orig_str(self, *a, **k)).decode()
    bass.Bass.to_json_bytes = lambda self, *a, **k: _rewrite(
        orig_bytes(self, *a, **k))
    bass.Bass._wait_split_installed = True


# ---------------------------------------------------------------------------
# Device program
# ---------------------------------------------------------------------------


def _build_bass(seq: int = SEQ, unroll_loop: bool = False,
                passes: int = PASSES) -> bass.Bass:
    from contextlib import ExitStack

    nc = bass.Bass()
    x0T = nc.declare_dram_parameter("x0T", [128, KE, BL], FP8, isOutput=False)
    h0T = nc.declare_dram_parameter("h0T", [H, BL], BF16, isOutput=False)
    c0T = nc.declare_dram_parameter("c0T", [H, BL], BF16, isOutput=False)
    wih8 = nc.declare_dram_parameter("wih8", [128, KE, 4 * H], FP8, isOutput=False)
    whhT = nc.declare_dram_parameter("whhT", [H, 4 * H], BF16, isOutput=False)
    fcw8d = nc.declare_dram_parameter("fcw8", [128, KH, E], FP8, isOutput=False)
    biasv = nc.declare_dram_parameter("biasv", [4 * H], F32, isOutput=False)
    wdiff = nc.declare_dram_parameter("wdiff", [H], BF16, isOutput=False)
    biasd = nc.declare_dram_parameter("biasd", [1, 2], F32, isOutput=False)
    ident = nc.declare_dram_parameter("ident", [128, 128], BF16, isOutput=False)
    out = nc.declare_dram_parameter("out", [2, BL], F32, isOutput=True)

    gates = ("i", "f", "g", "o")
    gate_fn = {"i": AF.Sigmoid, "f": AF.Sigmoid, "g": AF.Tanh, "o": AF.Sigmoid}

    with tile.TileContext(nc) as tc, ExitStack() as gctx:
        const = gctx.enter_context(tc.tile_pool(name="const", bufs=1))
        bias_sb = const.tile([128, MG], F32, name="bias_sb", tag="bias_sb")
        nc.sync.dma_start(out=bias_sb, in_=biasv[:].rearrange("(m p) -> p m", p=128))
        wd_sb = const.tile([128, KH], BF16, name="wd_sb", tag="wd_sb")
        nc.sync.dma_start(out=wd_sb, in_=wdiff[:].rearrange("(k p) -> p k", p=128))
        bd_sb = const.tile([1, 2], F32, name="bd_sb", tag="bd_sb")
        nc.sync.dma_start(out=bd_sb, in_=biasd[:, :])
        id_sb = const.tile([128, 128], BF16, name="id_sb", tag="id_sb")
        nc.sync.dma_start(out=id_sb, in_=ident[:, :])

        for p in range(passes):
            bs = slice(p * BP, (p + 1) * BP)
            with ExitStack() as pctx:
                # --- pass-resident state ---
                ghp = pctx.enter_context(tc.tile_pool(name=f"gh{p}", bufs=1))
                c0p = pctx.enter_context(tc.tile_pool(name=f"c0{p}", bufs=1))
                xp = pctx.enter_context(tc.tile_pool(name=f"x{p}", bufs=1))
                gh = [ghp.tile([128, BP], BF16, name=f"gh{p}_{m}", tag=f"gh{m}")
                      for m in range(MG)]
                c0t = [c0p.tile([128, BP], BF16, name=f"c0{p}_{j}", tag=f"c0{j}")
                       for j in range(KH)]
                xt = xp.tile([128, KE, BP], FP8, name=f"x{p}", tag="x")
                # --- phase B: gh' = W_hh @ h0T + bias (scoped: frees W_hh) ---
                with ExitStack() as bctx:
                    whhp = bctx.enter_context(tc.tile_pool(name=f"whh{p}", bufs=1))
                    h0p = bctx.enter_context(tc.tile_pool(name=f"h0{p}", bufs=1))
                    pghp = bctx.enter_context(
                        tc.tile_pool(name=f"pgh{p}", bufs=1, space="PSUM"))
                    whh_sb = [whhp.tile([128, 4 * H], BF16, name=f"whh{p}_{k}",
                                        tag=f"whh{k}") for k in range(KH)]
                    h0_sb = [h0p.tile([128, BP], BF16, name=f"h0{p}_{k}",
                                      tag=f"h0{k}") for k in range(KH)]
                    HALF = 2 * H
                    for k in range(KH):
                        nc.sync.dma_start(out=whh_sb[k][:, :HALF],
                                          in_=whhT[k * 128:(k + 1) * 128, :HALF])
                        nc.gpsimd.dma_start(out=whh_sb[k][:, HALF:],
                                            in_=whhT[k * 128:(k + 1) * 128, HALF:])
                        (nc.sync if k % 2 else nc.gpsimd).dma_start(
                            out=h0_sb[k], in_=h0T[k * 128:(k + 1) * 128, bs])
                    for j in range(KH):
                        nc.sync.dma_start(out=c0t[j],
                                          in_=c0T[j * 128:(j + 1) * 128, bs])
                    nc.sync.dma_start(out=xt, in_=x0T[:, :, bs])
                    for m in range(MG):
                        ps = pghp.tile([128, BP], F32, name=f"pgh{p}_{m}",
                                       tag="pgh", bufs=4)
                        for k in range(KH):
                            for n in range(NB):
                                nc.tensor.matmul(
                                    ps[:, n * NT:(n + 1) * NT],
                                    lhsT=whh_sb[k][:, m * 128:(m + 1) * 128],
                                    rhs=h0_sb[k][:, n * NT:(n + 1) * NT],
                                    start=(k == 0), stop=(k == KH - 1))
                        if m % 2 == 0:
                            nc.vector.tensor_copy(gh[m], ps)
                        else:
                            nc.scalar.activation(gh[m], ps, AF.Copy)

                # --- main pools ---
                wp = pctx.enter_context(tc.tile_pool(name=f"wih{p}", bufs=1))
                fp_ = pctx.enter_context(tc.tile_pool(name=f"fcw{p}", bufs=1))
                hp = pctx.enter_context(tc.tile_pool(name=f"h{p}", bufs=1))
                work = pctx.enter_context(tc.tile_pool(name=f"work{p}", bufs=2))
                ps1p = pctx.enter_context(
                    tc.tile_pool(name=f"ps1{p}", bufs=2, space="PSUM"))
                ps2p = pctx.enter_context(
                    tc.tile_pool(name=f"ps2{p}", bufs=2, space="PSUM"))

                wih_sb = wp.tile([128, KE, 4 * H], FP8, name=f"wih{p}",
                                 tag="wih")
                fcw8 = fp_.tile([128, KH, E], FP8, name=f"fcw{p}", tag="fcw")
                h_sb = [hp.tile([128, BP], BF16, name=f"h{p}_{j}", tag=f"h{j}")
                        for j in range(KH)]
                h8 = hp.tile([128, KH, BP], FP8, name=f"h8{p}", tag="h8")
                nc.sync.dma_start(out=wih_sb, in_=wih8[:, :, :])
                nc.sync.dma_start(out=fcw8, in_=fcw8d[:, :, :])

                # --- 64-step recurrence ---
                # Emission is software-pipelined: the elementwise c/h chain
                # for slice j-1 is emitted between slice j's gate groups so
                # the static per-engine instruction order never stalls on a
                # cross-engine dependency that was issued immediately before.
                # Gate order (i,g,f,o) lets t2 = sig(i)*tanh(g) fire early.
                gorder = ("i", "g", "f", "o")

                def emit_gates(j, pend=(), after=None, sig=None):
                    # gh is preloaded into PSUM by an identity matmul (PE)
                    # and the sigma/tanh reads PSUM directly with the bias
                    # applied on the Activation engine.
                    pend = list(pend)
                    if sig is None:
                        sig = {}
                    for g in gorder:
                        if pend:
                            pend.pop(0)()
                        m = gates.index(g) * KH + j
                        ps = ps1p.tile([128, BP], F32, name=f"ps1_{j}{g}",
                                       tag="ps1", bufs=2)
                        for n in range(NB):
                            nc.tensor.matmul(
                                ps[:, n * NT:(n + 1) * NT],
                                lhsT=id_sb,
                                rhs=gh[m][:, n * NT:(n + 1) * NT],
                                start=True, stop=False)
                        for s in range(0, KE, 2):
                            for n in range(NB):
                                nc.tensor.matmul(
                                    ps[:, n * NT:(n + 1) * NT],
                                    lhsT=wih_sb[:, s:s + 2,
                                                m * 128:(m + 1) * 128],
                                    rhs=xt[:, s:s + 2, n * NT:(n + 1) * NT],
                                    start=False,
                                    stop=(s == KE - 2),
                                    perf_mode=DR)
                        s = work.tile([128, BP], BF16, name=f"sig_{j}{g}",
                                      tag=f"sig{g}", bufs=4)
                        nc.scalar.activation(s, ps, gate_fn[g],
                                             bias=bias_sb[:, m:m + 1])
                        sig[g] = s
                        if after and g in after:
                            after[g]()
                    for piece in pend:
                        piece()
                    return sig

                def chain_tiles(j):
                    t1 = work.tile([128, BP], BF16, name=f"t1_{j}",
                                   tag="t1", bufs=3)
                    t2 = work.tile([128, BP], BF16, name=f"t2_{j}",
                                   tag="t2", bufs=3)
                    cc = work.tile([128, BP], BF16, name=f"cc_{j}",
                                   tag="cc", bufs=3)
                    tch = work.tile([128, BP], BF16, name=f"tch_{j}",
                                    tag="tch", bufs=3)
                    return t1, t2, cc, tch

                def cpath_pieces(j, sig, last=False):
                    """Yield the c/h chain for slice j as 4 pieces, to be
                    interleaved between the next slice's gate groups so no
                    engine's in-order stream stalls on a fresh dependency."""
                    t1, t2, cc, tch = chain_tiles(j)

                    def p0():
                        nc.vector.tensor_mul(t2, sig["i"], sig["g"])

                    def p1():
                        nc.vector.tensor_mul(t1, sig["f"], c0t[j])

                    def p2():
                        nc.vector.tensor_add(cc, t1, t2)
                        nc.scalar.activation(tch, cc, AF.Tanh)

                    def p3():
                        # Single fused multiply straight to fp8 (the only
                        # in-loop consumer is the DoubleRow fc matmul);
                        # engines alternate by slice parity.
                        if last:
                            # final step: bf16 h for the classifier head,
                            # no fc follows.
                            (nc.vector if j % 2 else
                             nc.gpsimd).tensor_mul(h_sb[j], sig["o"], tch)
                        elif j % 2:
                            nc.vector.tensor_mul(h8[:, j, :], sig["o"], tch)
                        else:
                            nc.gpsimd.tensor_mul(h8[:, j, :], sig["o"], tch)

                    return [p0, p1, p2, p3]

                def fc_pairs(ms, pairs, pss, first):
                    for i, m in enumerate(ms):
                        for k in pairs:
                            for n in range(NB):
                                nc.tensor.matmul(
                                    pss[i][:, n * NT:(n + 1) * NT],
                                    lhsT=fcw8[:, k:k + 2,
                                              m * 128:(m + 1) * 128],
                                    rhs=h8[:, k:k + 2, n * NT:(n + 1) * NT],
                                    start=(k == first),
                                    stop=(k == KH - 2),
                                    perf_mode=DR)

                def step_body(last=False):
                    if last:
                        # peeled final step: h in bf16 for the head, no fc /
                        # xt / h8 work at all.
                        pend = []
                        for j in range(KH):
                            sig = emit_gates(j, pend)
                            pend = cpath_pieces(j, sig, last=True)
                        for piece in pend:
                            piece()
                        return

                    pend = []
                    xparts = []
                    pssA = []
                    pss01 = []
                    sig7 = {}

                    def mk_xpart(i, m, src):
                        xp_ = work.tile([128, BP], BF16, name=f"xpart_{m}",
                                        tag=f"xpart{i}", bufs=1)
                        (nc.vector if i == 0 else nc.gpsimd).tensor_copy(
                            xp_, src)
                        xparts.append(xp_)

                    # j==7 runs its own c/h chain inline via `after` hooks so
                    # the serial tail (cc -> tanh -> h8 -> fc tails -> xt)
                    # starts as early as the dependencies allow.
                    t1_7, t2_7, cc_7, tch_7 = (None,) * 4

                    def j7_after_f():
                        nc.vector.tensor_mul(t2_7, sig7["i"], sig7["g"])
                        # fc pair (4,5) for m2/m3 (dep h8_5 landed at end of
                        # j6) + park both partials, freeing the PSUM slots.
                        fc_pairs((2, 3), (4,), pssA, None)
                        mk_xpart(0, 2, pssA[0])
                        mk_xpart(1, 3, pssA[1])

                    def j7_after_o():
                        nc.vector.tensor_mul(t1_7, sig7["f"], c0t[7])
                        nc.vector.tensor_add(cc_7, t1_7, t2_7)
                        nc.scalar.activation(tch_7, cc_7, AF.Tanh)
                        nc.vector.tensor_mul(h8[:, 7, :], sig7["o"], tch_7)
                        # wave B: m0/m1 over h-pairs (0..5) into the freed
                        # PSUM slots; PE is otherwise idle in this window.
                        pss01.extend(
                            ps2p.tile([128, BP], F32, name=f"ps2_{m}",
                                      tag="ps2", bufs=2) for m in (0, 1))
                        fc_pairs((0, 1), (0, 2, 4), pss01, 0)

                    for j in range(KH):
                        if j == 7:
                            t1_7, t2_7, cc_7, tch_7 = chain_tiles(7)
                            emit_gates(7, pend,
                                       after={"f": j7_after_f,
                                              "o": j7_after_o},
                                       sig=sig7)
                            break
                        sig = emit_gates(j, pend)
                        pend = cpath_pieces(j, sig)
                        if j == 5:
                            # wave A: m2/m3 over h-pairs (0..3) -- deps two
                            # slices old, no PE head-of-line risk.
                            pssA.extend(
                                ps2p.tile([128, BP], F32, name=f"ps2_{m}",
                                          tag="ps2", bufs=2) for m in (2, 3))
                            fc_pairs((2, 3), (0, 2), pssA, 0)
                    # tails: only the (h6,h7) pair + converts remain after
                    # the last h slice. xt0 lands on ACT (idle here), xt1 on
                    # DVE; the parked m2/m3 rejoin via adds on DVE/Pool.
                    fc_pairs((0, 1), (6,), pss01, None)
                    nc.scalar.activation(xt[:, 0, :], pss01[0], AF.Copy)
                    nc.vector.tensor_copy(xt[:, 1, :], pss01[1])
                    pssB = [ps2p.tile([128, BP], F32, name=f"ps2b_{m}",
                                      tag="ps2", bufs=2) for m in (2, 3)]
                    for i, m in enumerate((2, 3)):
                        for n in range(NB):
                            nc.tensor.matmul(
                                pssB[i][:, n * NT:(n + 1) * NT],
                                lhsT=fcw8[:, 6:8, m * 128:(m + 1) * 128],
                                rhs=h8[:, 6:8, n * NT:(n + 1) * NT],
                                start=True, stop=True,
                                perf_mode=DR)
                    nc.vector.tensor_add(xt[:, 2, :], pssB[0], xparts[0])
                    nc.gpsimd.tensor_add(xt[:, 3, :], pssB[1], xparts[1])

                if unroll_loop:
                    for t in range(seq):
                        step_body(last=(t == seq - 1))
                else:
                    assert seq % 8 == 0 and seq >= 16
                    with tc.For_i(0, seq - 8, 8,
                                  hint_engines=(mybir.EngineType.PE,
                                                mybir.EngineType.DVE,
                                                mybir.EngineType.Activation,
                                                mybir.EngineType.Pool)):
                        for _ in range(8):
                            step_body()
                    for t in range(8):
                        step_body(last=(t == 7))

                # --- head: d = wdiff @ h_63; p0 = sig(d+bd), p1 = sig(-d-bd) ---
                psd = ps2p.tile([1, BP], F32, name=f"psd{p}", tag="ps2", bufs=2)
                for n in range(NB):
                    for k in range(KH):
                        nc.tensor.matmul(
                            psd[0:1, n * NT:(n + 1) * NT],
                            lhsT=wd_sb[:, k:k + 1],
                            rhs=h_sb[k][:, n * NT:(n + 1) * NT],
                            start=(k == 0), stop=(k == KH - 1))
                p0 = work.tile([1, BP], F32, name=f"p0_{p}", tag="p0", bufs=1)
                p1 = work.tile([1, BP], F32, name=f"p1_{p}", tag="p1", bufs=1)
                nc.scalar.activation(p0, psd, AF.Sigmoid,
                                     bias=bd_sb[0:1, 0:1], scale=1.0)
                nc.scalar.activation(p1, psd, AF.Sigmoid,
                                     bias=bd_sb[0:1, 1:2], scale=-1.0)
                nc.sync.dma_start(out=out[0:1, bs], in_=p0)
                nc.sync.dma_start(out=out[1:2, bs], in_=p1)
    return nc


# ---------------------------------------------------------------------------
# Host wrapper
# ---------------------------------------------------------------------------


def kernel(start_emb, h0, c0, W_ih, W_hh, b_ih, b_hh, fc_W, fc_b,
           final_W, final_b):
    _install_wait_split_hook()

    start_emb = np.asarray(start_emb, np.float32)
    h0 = np.asarray(h0, np.float32)
    c0 = np.asarray(c0, np.float32)
    W_ih = np.asarray(W_ih, np.float32)
    W_hh = np.asarray(W_hh, np.float32)
    b_ih = np.asarray(b_ih, np.float32)
    b_hh = np.asarray(b_hh, np.float32)
    fc_W = np.asarray(fc_W, np.float32)
    fc_b = np.asarray(fc_b, np.float32)
    final_W = np.asarray(final_W, np.float32)
    final_b = np.asarray(final_b, np.float32)

    # shared (replicated) weight prep, all layout work on host
    wih8 = np.ascontiguousarray(
        W_ih.T.reshape(KE, 128, 4 * H).transpose(1, 0, 2)).astype(NPF8)
    whhT = np.ascontiguousarray(W_hh.T).astype(NPBF)              # [H, 4H]
    fcw8 = np.ascontiguousarray(
        fc_W.T.reshape(KH, 128, E).transpose(1, 0, 2)).astype(NPF8)
    biasv = (b_ih + b_hh + W_ih @ fc_b).astype(np.float32)        # [4H]
    wdiff = (final_W[0] - final_W[1]).astype(NPBF)                # [H]
    bd = float(final_b[0]) - float(final_b[1])
    biasd = np.array([[bd, -bd]], np.float32)
    identity = np.eye(128, dtype=NPBF)

    x0 = start_emb[:, 0, :] - fc_b                                # [B, E]
    x0T8 = np.ascontiguousarray(
        x0.T.reshape(KE, 128, B).transpose(1, 0, 2)).astype(NPF8)
    h0s = h0[0]                                                   # [B, H]
    c0s = c0[0]                                                   # [B, H]

    in_maps = []
    for ci in range(N_CORES):
        sl = slice(ci * BL, (ci + 1) * BL)
        in_maps.append({
            "x0T": np.ascontiguousarray(x0T8[:, :, sl]),
            "h0T": np.ascontiguousarray(h0s[sl].T).astype(NPBF),
            "c0T": np.ascontiguousarray(c0s[sl].T).astype(NPBF),
            "wih8": wih8,
            "whhT": whhT,
            "fcw8": fcw8,
            "biasv": biasv,
            "wdiff": wdiff,
            "biasd": biasd,
            "ident": identity,
        })

    nc = _build_bass()
    kernel.last_nc = nc
    import time as _time
    t0 = _time.monotonic()
    res = run_bass_kernel_spmd(nc, in_maps, list(range(N_CORES)),
                               trace=TRACE, **TRACE_KWARGS)
    kernel.last_wall_s = _time.monotonic() - t0
    kernel.last_results = res

    full = np.empty((B, 1, 2), np.float32)
    for ci in range(N_CORES):
        o = res.results[ci]["out"]                                # [2, BL]
        full[ci * BL:(ci + 1) * BL, 0, 0] = o[0]
        full[ci * BL:(ci + 1) * BL, 0, 1] = o[1]
    return full
